# revision 21
# baseline (speedup 1.0000x reference)
"""Binarized-CNN BasicBlock (2x conv3x3 256ch + train-mode BN + hardtanh +
residual) on 8 trn2 NeuronCores, data-parallel over the batch.

Key structure:
  - binarize(x) in {-1,+1} stored as bf16 -> conv = exact integer sums in
    fp32 PSUM via 18 accumulating matmuls (2 channel tiles x 9 taps) over a
    zero-padded 30x30 spatial layout.
  - conv bias b1/b2 cancel under training-mode BN (shift invariance) and are
    never applied.
  - sign(hardtanh(bn(v))) == sign(v*scale + bias) so conv2's input needs only
    an affine threshold of conv1's raw output.
  - BN statistics span the full batch: per-core partial sum/sumsq are
    all-reduced across the 8 cores (one tiny [128,4] f32 collective per BN).
"""

import numpy as np
import ml_dtypes

import concourse.bacc as bacc
import concourse.tile as tile
from concourse import mybir
from concourse.bass_utils import run_bass_kernel_spmd

# ---------------- problem constants (hardcoded) ----------------
N_CORES = 8
N_FULL, C, H, W = 128, 256, 28, 28
NPC = N_FULL // N_CORES          # 16 images per core
HP = WP = 30                     # padded frame
IMG = HP * WP                    # 900
NPOS = NPC * IMG                 # 14400 padded positions per core
MARG = 32                        # margin so shifted reads stay in-bounds
BUF = NPOS + 2 * MARG            # 14464
SLAB = 450                       # matmul moving free dim (15 padded rows)
NSLABS = NPOS // SLAB            # 32
GROUP = 4                        # psum tiles per accumulation group
NGROUPS = NSLABS // GROUP        # 8
NTOT = N_FULL * H * W            # 100352 samples per channel (full batch)
EPS = 1e-5
QI = 2                           # images per streaming chunk
NCHUNK = NPC // QI               # 8 chunks

BF16 = mybir.dt.bfloat16
F32 = mybir.dt.float32
AF = mybir.ActivationFunctionType
ALU = mybir.AluOpType


def _interior(buf, a, q):
    """[128, q, 28, 28] view of the valid pixels of images a..a+q in a padded
    [128, BUF] buffer."""
    v = buf[:, MARG + a * IMG: MARG + (a + q) * IMG]
    v = v.rearrange("p (n r c) -> p n r c", r=HP, c=WP)
    return v[:, :, 1:29, 1:29]


def _x_dram_ap(xd, a, q, mi):
    """DRAM AP for images a..a+q, channel tile mi -> [128, q, 784]."""
    sl = xd[a: a + q, mi * 128: (mi + 1) * 128, :, :]
    return sl.rearrange("n c h w -> c n (h w)")


def _build(stage=4):
    nc = bacc.Bacc(
        "TRN2", target_bir_lowering=False, debug=False, num_devices=N_CORES
    )
    xd = nc.dram_tensor("x", [NPC, C, H, W], F32, kind="ExternalInput")
    w1d = nc.dram_tensor("w1s", [2, 128, 9 * 256], BF16, kind="ExternalInput")
    w2d = nc.dram_tensor("w2s", [2, 128, 9 * 256], BF16, kind="ExternalInput")
    bnd = nc.dram_tensor("bnp", [128, 8], F32, kind="ExternalInput")
    yd = nc.dram_tensor("y", [NPC, C, H, W], F32, kind="ExternalOutput")

    # tap offsets in the padded layout
    doff = [(dy - 1) * WP + (dx - 1) for dy in range(3) for dx in range(3)]

    with tile.TileContext(nc) as tc:
        with (
            tc.tile_pool(name="wp", bufs=1) as wp,
            tc.tile_pool(name="xsp", bufs=1) as xsp,
            tc.tile_pool(name="vp", bufs=1) as vp,
            tc.tile_pool(name="small", bufs=1) as small,
            tc.tile_pool(name="instage", bufs=3) as instage,
            tc.tile_pool(name="chunks", bufs=2) as chunks,
            tc.tile_pool(name="scrp", bufs=1) as scrp,
            tc.tile_pool(name="psum", bufs=8, space="PSUM") as psum,
            tc.tile_pool(name="dram", bufs=1, space="DRAM") as dram,
        ):
            # ---- load weights & bn params ----
            wsb = {}
            wconvs = ((1, w1d), (2, w2d)) if stage >= 3 else ((1, w1d),)
            for conv, wd in wconvs:
                for ci in range(2):
                    t = wp.tile([128, 9 * 256], BF16, tag=f"w{conv}_{ci}")
                    nc.sync.dma_start(out=t[:], in_=wd[ci])
                    wsb[(conv, ci)] = t
            if stage >= 2:
                bnp = small.tile([128, 8], F32, tag="bnp")
                nc.sync.dma_start(out=bnp[:], in_=bnd[:])
                eps_sb = small.tile([128, 1], F32, tag="eps")
                nc.vector.memset(eps_sb[:], EPS)

            # ---- sign-input buffers (zeroed: pads/margins must be 0) ----
            xs = []
            for ci in range(2):
                t = xsp.tile([128, BUF], BF16, tag=f"xs{ci}")
                nc.vector.memset(t[:], 0.0)
                xs.append(t)
            v = [
                vp.tile([128, BUF], BF16, name=f"v{ci}", tag=f"v{ci}")
                for ci in range(2)
            ]

            # ---- load x, binarize into padded layout ----
            for ci in range(2):
                for k in range(NCHUNK):
                    a = k * QI
                    st = instage.tile([128, QI * 784], F32, tag="xin")
                    nc.sync.dma_start(
                        out=st.rearrange("p (n f) -> p n f", n=QI),
                        in_=_x_dram_ap(xd, a, QI, ci),
                    )
                    nc.scalar.activation(
                        out=_interior(xs[ci], a, QI),
                        in_=st.rearrange("p (n r c) -> p n r c", r=28, c=28),
                        func=AF.Sign,
                    )

            def conv(idx, src, dst):
                """dst[m] (padded bf16) = conv3x3(src binarized input)."""
                for m in range(2):
                    for g in range(NGROUPS):
                        ps = [
                            psum.tile([128, SLAB], F32, name=f"ps{m}_{g}_{i}", tag="ps")
                            for i in range(GROUP)
                        ]
                        for ci in range(2):
                            for off in range(9):
                                lhsT = wsb[(idx, ci)][
                                    :, off * 256 + m * 128: off * 256 + m * 128 + 128
                                ]
                                for s4 in range(GROUP):
                                    s = g * GROUP + s4
                                    base = MARG + s * SLAB + doff[off]
                                    nc.tensor.matmul(
                                        ps[s4][:],
                                        lhsT,
                                        src[ci][:, base: base + SLAB],
                                        start=(ci == 0 and off == 0),
                                        stop=(ci == 1 and off == 8),
                                    )
                        for s4 in range(GROUP):
                            s = g * GROUP + s4
                            nc.vector.tensor_copy(
                                out=dst[m][:, MARG + s * SLAB: MARG + (s + 1) * SLAB],
                                in_=ps[s4][:],
                            )

            def readout(bufs):
                """debug: copy padded-bf16 buffer interiors to y."""
                for m in range(2):
                    for k in range(NCHUNK):
                        a = k * QI
                        oc = chunks.tile([128, QI * 784], F32, tag="oc")
                        nc.vector.tensor_copy(
                            out=oc.rearrange("p (n r c) -> p n r c", r=28, c=28),
                            in_=_interior(bufs[m], a, QI),
                        )
                        nc.sync.dma_start(
                            out=_x_dram_ap(yd, a, QI, m),
                            in_=oc.rearrange("p (n f) -> p n f", n=QI),
                        )

            def allreduce_stats(stat_tile, name):
                in_b = dram.tile([128, 4], F32, tag=f"arin{name}")
                out_b = dram.tile([128, 4], F32, tag=f"arout{name}")
                red = small.tile([128, 4], F32, tag=f"red{name}")
                nc.sync.dma_start(out=in_b[:], in_=stat_tile[:])
                nc.gpsimd.collective_compute(
                    "AllReduce",
                    ALU.add,
                    replica_groups=[list(range(N_CORES))],
                    ins=[in_b.opt()],
                    outs=[out_b.opt()],
                )
                nc.sync.dma_start(out=red[:], in_=out_b[:])
                return red

            def bn_coeffs(red, layer, name):
                """scale = gamma*rsqrt(var+eps); bias = beta - mean*scale."""
                scales, biases = [], []
                for m in range(2):
                    mean = small.tile([128, 1], F32, tag=f"mean{name}{m}")
                    nc.vector.tensor_scalar_mul(mean[:], red[:, m: m + 1], 1.0 / NTOT)
                    ex2 = small.tile([128, 1], F32, tag=f"ex2{name}{m}")
                    nc.vector.tensor_scalar_mul(ex2[:], red[:, 2 + m: 3 + m], 1.0 / NTOT)
                    var = small.tile([128, 1], F32, tag=f"var{name}{m}")
                    nc.vector.tensor_tensor(
                        out=var[:], in0=mean[:], in1=mean[:], op=ALU.mult
                    )
                    nc.vector.tensor_tensor(
                        out=var[:], in0=ex2[:], in1=var[:], op=ALU.subtract
                    )
                    std = small.tile([128, 1], F32, tag=f"std{name}{m}")
                    nc.scalar.activation(
                        out=std[:], in_=var[:], func=AF.Sqrt, bias=eps_sb[:]
                    )
                    inv = small.tile([128, 1], F32, tag=f"inv{name}{m}")
                    nc.vector.reciprocal(out=inv[:], in_=std[:])
                    gcol = 4 * m if layer == 1 else 4 * m + 2
                    bcol = gcol + 1
                    sc = small.tile([128, 1], F32, tag=f"sc{name}{m}")
                    nc.vector.tensor_tensor(
                        out=sc[:], in0=inv[:], in1=bnp[:, gcol: gcol + 1], op=ALU.mult
                    )
                    bi = small.tile([128, 1], F32, tag=f"bi{name}{m}")
                    nc.vector.tensor_tensor(
                        out=bi[:], in0=mean[:], in1=sc[:], op=ALU.mult
                    )
                    nc.vector.tensor_tensor(
                        out=bi[:], in0=bnp[:, bcol: bcol + 1], in1=bi[:],
                        op=ALU.subtract,
                    )
                    scales.append(sc)
                    biases.append(bi)
                return scales, biases

            # ================= conv1 =================
            conv(1, xs, v)
            if stage == 1:
                readout(v)

            if stage >= 2:
                # ---- BN1 stats: per-channel sum & sumsq of raw conv1 ----
                stat_sb = small.tile([128, 4], F32, tag="stat1")
                qac = small.tile([128, 2, NCHUNK], F32, tag="qac1")
                for m in range(2):
                    nc.vector.reduce_sum(
                        out=stat_sb[:, m: m + 1],
                        in_=_interior(v[m], 0, NPC),
                        axis=mybir.AxisListType.XYZ,
                    )
                    for k in range(NCHUNK):
                        scr = scrp.tile([128, QI * 784], F32, tag="scr")
                        nc.scalar.activation(
                            out=scr.rearrange("p (n r c) -> p n r c", r=28, c=28),
                            in_=_interior(v[m], k * QI, QI),
                            func=AF.Square,
                            accum_out=qac[:, m, k: k + 1],
                        )
                for m in range(2):
                    nc.vector.reduce_sum(
                        out=stat_sb[:, 2 + m: 3 + m],
                        in_=qac[:, m, :],
                        axis=mybir.AxisListType.X,
                    )

                red1 = allreduce_stats(stat_sb, "1")
                sc1, bi1 = bn_coeffs(red1, 1, "1")

                # ---- conv2 input: sign(v1*scale+bias) into xs (pads stay 0) --
                for m in range(2):
                    nc.scalar.activation(
                        out=_interior(xs[m], 0, NPC),
                        in_=_interior(v[m], 0, NPC),
                        func=AF.Sign,
                        bias=bi1[m][:],
                        scale=sc1[m][:],
                    )
                if stage == 2:
                    readout(xs)

            if stage >= 3:
                # ================= conv2 =================
                conv(2, xs, v)
                if stage == 3:
                    readout(v)

            if stage < 4:
                nc_finish = True
            else:
                nc_finish = _tail(nc, tc, xd, yd, v, small, chunks, scrp,
                                  allreduce_stats, bn_coeffs)
    nc.compile()
    # pass-ordering bug in this bacc vintage: late compile passes can leave
    # >1 sync wait on an instruction (HW cap); one more split pass fixes it
    nc.generate_event_semaphores()
    return nc


def _tail(nc, tc, xd, yd, v, small, chunks, scrp, allreduce_stats, bn_coeffs):
            # ---- BN2 stats over t2 = conv2 + residual ----
            stat2 = small.tile([128, 4], F32, tag="stat2")
            sac2 = small.tile([128, 2, NCHUNK], F32, tag="sac2")
            qac2 = small.tile([128, 2, NCHUNK], F32, tag="qac2")
            for m in range(2):
                for k in range(NCHUNK):
                    a = k * QI
                    resc = chunks.tile([128, QI * 784], F32, tag="resc")
                    nc.sync.dma_start(
                        out=resc.rearrange("p (n f) -> p n f", n=QI),
                        in_=_x_dram_ap(xd, a, QI, m),
                    )
                    t2c = chunks.tile([128, QI * 784], F32, tag="t2c")
                    nc.vector.tensor_tensor(
                        out=t2c.rearrange("p (n r c) -> p n r c", r=28, c=28),
                        in0=_interior(v[m], a, QI),
                        in1=resc.rearrange("p (n r c) -> p n r c", r=28, c=28),
                        op=ALU.add,
                    )
                    nc.vector.reduce_sum(
                        out=sac2[:, m, k: k + 1], in_=t2c[:],
                        axis=mybir.AxisListType.X,
                    )
                    scr = scrp.tile([128, QI * 784], F32, tag="scr")
                    nc.scalar.activation(
                        out=scr[:],
                        in_=t2c[:],
                        func=AF.Square,
                        accum_out=qac2[:, m, k: k + 1],
                    )
            for m in range(2):
                nc.vector.reduce_sum(
                    out=stat2[:, m: m + 1], in_=sac2[:, m, :], axis=mybir.AxisListType.X
                )
                nc.vector.reduce_sum(
                    out=stat2[:, 2 + m: 3 + m], in_=qac2[:, m, :],
                    axis=mybir.AxisListType.X,
                )

            red2 = allreduce_stats(stat2, "2")
            sc2, bi2 = bn_coeffs(red2, 2, "2")

            # ---- final: y = clip(t2*scale + bias, -1, 1) ----
            for m in range(2):
                for k in range(NCHUNK):
                    a = k * QI
                    resc = chunks.tile([128, QI * 784], F32, tag="resc")
                    nc.sync.dma_start(
                        out=resc.rearrange("p (n f) -> p n f", n=QI),
                        in_=_x_dram_ap(xd, a, QI, m),
                    )
                    t2c = chunks.tile([128, QI * 784], F32, tag="t2c")
                    nc.vector.tensor_tensor(
                        out=t2c.rearrange("p (n r c) -> p n r c", r=28, c=28),
                        in0=_interior(v[m], a, QI),
                        in1=resc.rearrange("p (n r c) -> p n r c", r=28, c=28),
                        op=ALU.add,
                    )
                    u = chunks.tile([128, QI * 784], F32, tag="u")
                    nc.scalar.activation(
                        out=u[:], in_=t2c[:], func=AF.Identity,
                        bias=bi2[m][:], scale=sc2[m][:],
                    )
                    oc = chunks.tile([128, QI * 784], F32, tag="oc")
                    nc.vector.tensor_scalar(
                        out=oc[:], in0=u[:],
                        scalar1=-1.0, scalar2=1.0,
                        op0=ALU.max, op1=ALU.min,
                    )
                    nc.sync.dma_start(
                        out=_x_dram_ap(yd, a, QI, m),
                        in_=oc.rearrange("p (n f) -> p n f", n=QI),
                    )
            return True


_NC_CACHE = None


def _get_nc():
    global _NC_CACHE
    if _NC_CACHE is None:
        _NC_CACHE = _build()
    return _NC_CACHE


def _prep_weights(w):
    """[O=256,I=256,3,3] f32 -> sign-binarized lhsT layout [2,128,9*256] bf16:
    [ci][c][off*256+o] = sign(w[o, ci*128+c, off])."""
    ws = np.sign(np.asarray(w, dtype=np.float32))
    ws = ws.reshape(256, 2, 128, 9).transpose(1, 2, 3, 0).reshape(2, 128, 9 * 256)
    return np.ascontiguousarray(ws.astype(ml_dtypes.bfloat16))


def kernel(x, w1, b1, g1, be1, w2, b2, g2, be2):
    x = np.asarray(x, dtype=np.float32)
    w1s = _prep_weights(w1)
    w2s = _prep_weights(w2)
    # per-partition BN params: [128, 8] cols = (g1,be1,g2,be2) for m=0, then m=1
    bnp = np.stack(
        [
            np.asarray(g1, np.float32).reshape(2, 128),
            np.asarray(be1, np.float32).reshape(2, 128),
            np.asarray(g2, np.float32).reshape(2, 128),
            np.asarray(be2, np.float32).reshape(2, 128),
        ],
        axis=-1,
    )  # [2, 128, 4]
    bnp = np.ascontiguousarray(bnp.transpose(1, 0, 2).reshape(128, 8))

    nc = _get_nc()
    in_maps = [
        {
            "x": np.ascontiguousarray(x[c * NPC: (c + 1) * NPC]),
            "w1s": w1s,
            "w2s": w2s,
            "bnp": bnp,
        }
        for c in range(N_CORES)
    ]
    res = run_bass_kernel_spmd(nc, in_maps, list(range(N_CORES)))
    return np.concatenate([res.results[c]["y"] for c in range(N_CORES)], axis=0)
